# revision 4
# baseline (speedup 1.0000x reference)
"""LocalGaussianBlur v3 — Trainium2 Bass kernel (7x7 truncation, DVE+GPSIMD).

Math: sigma = modulator[h,w] in (0,1); u = 1/(2 sigma^2 + 1e-8);
q = exp(-u) <= exp(-0.5).  Weight of tap (j,t) is q^(j^2+t^2).
Since q <= 0.6065, taps with |j| or |t| >= 4 carry < 5.4e-4 of the
kernel mass; normalizing by the truncated sum s3 = 1 + 2(q+q^4+q^9)
makes the truncated kernel a proper weighted average.  Rings kept:
m in {1,2,4,5,8,9,10} (13/18 dropped).

out = [Xc + q C1 + q^2 C2 + q^4 C4 + q^5 C5 + q^8 C8 + q^9 C9
        + q^10 C10] / s3^2

Layout per core (8-way H-shard, 64 rows each):
  partitions p = rq*32+cb (4 row-quarters x 32 col-blocks),
  per-partition spatial block 16x16, X with halo rows 3 / cols 4
  -> X tile [128, 3ch, 22, 24] bf16.  XO is a one-column-shifted copy
  keeping every column-pair add 4B-aligned for the DVE 2x bf16 mode.
  Weight maps are per-pixel [128,16,16], broadcast over the channel
  axis with stride-0 APs.

v3: channel-parallel engine split.  The three RGB channels share every
per-pixel weight map and all ops are channel-local, so channel 2 of the
combine suffix (PR -> L1 -> W -> F -> OUT, the biggest DVE stretch)
runs on GPSIMD (nc.gpsimd.tensor_tensor, 8 Q7 DSPs) in parallel with
DVE doing channels 0-1.  Single-channel blocks are fully contiguous ->
every GPSIMD AP collapses to [P, slots, rows*cols] (<=2 free dims, the
Pool TENSOR3D limit).  GPSIMD writes PRIVATE tiles so the dependency
tracker cannot serialize the engines.  Measured: GPSIMD TT = 1.91
ns/elem, running at ~54% of that while DVE TT is active (shared-port
arbitration) — sized so GPSIMD finishes just before DVE.
LGB2_GPN: 0 = all-DVE baseline, 1 = ch2 suffix on GPSIMD,
2 = also the A-stage ch2 on GPSIMD (shared-tile write).
"""

import os
import numpy as np

H = W = 512
C = 3
NC = 8
RS = H // NC        # 64 rows per core
RQ = 4              # row-quarters per core
CB = 32             # col blocks
TR = 16             # block rows
TC = 16             # block cols
RHL = 3             # row halo
CHL = 4             # col halo (even => aligned bf16 slices)
XR = TR + 2 * RHL   # 22
XC = TC + 2 * CHL   # 24
P = 128

_NC_CACHE = {}


def _gpn():
    return int(os.environ.get("LGB2_GPN", "1"))


def _build_nc():
    key = ("nc", os.environ.get("LGB2_REPEAT", "1"), _gpn())
    if key in _NC_CACHE:
        return _NC_CACHE[key]
    import concourse.bass as bass  # noqa: F401
    from concourse import bacc
    import concourse.mybir as mybir
    from concourse.tile import TileContext
    from concourse.ap import AP as BassAP

    f32 = mybir.dt.float32
    bf16 = mybir.dt.bfloat16
    AF = mybir.ActivationFunctionType
    ALU = mybir.AluOpType

    nc = bacc.Bacc()
    x = nc.dram_tensor("x", [P, C, XR, XC], bf16, kind="ExternalInput")
    xo = nc.dram_tensor("xo", [P, C, XR, XC], bf16, kind="ExternalInput")
    xn = nc.dram_tensor("xn", [P, C, XR, TC], bf16, kind="ExternalInput")
    md = nc.dram_tensor("md", [P, TR, TC], f32, kind="ExternalInput")
    out = nc.dram_tensor("out", [P, C, TR, TC], f32, kind="ExternalOutput")
    outg = nc.dram_tensor("outg", [P, TR, TC], f32, kind="ExternalOutput")

    nrep = int(os.environ.get("LGB2_REPEAT", "1"))
    GPN = _gpn()
    CD = 2 if GPN > 0 else C   # channels DVE covers in the combine suffix
    GC = C - 1                 # the channel GPSIMD owns

    with TileContext(nc) as tc:
        with tc.tile_pool(name="main", bufs=1) as pool:
            X = pool.tile([P, C, XR, XC], bf16, tag="X")
            XO = pool.tile([P, C, XR, XC], bf16, tag="XO")
            # XN: center cols only (no col halo) -> rows are contiguous, so
            # slot-strided multi-row-pair APs collapse to 3 free dims
            XN = pool.tile([P, C, XR, TC], bf16, tag="XN")
            MD = pool.tile([P, TR, TC], f32, tag="MD")
            nc.sync.dma_start(out=MD[:], in_=md[:])
            nc.sync.dma_start(out=X[:], in_=x[:])
            nc.sync.dma_start(out=XO[:], in_=xo[:])
            nc.sync.dma_start(out=XN[:], in_=xn[:])
            EPS = pool.tile([P, 1], f32, tag="EPS")
            nc.vector.memset(EPS[:], 1e-4)

            # all tiles allocated up-front so the compute body can sit
            # inside a hardware loop (repeat-timing mode)
            V = pool.tile([P, TR, TC], f32, tag="V")
            U = pool.tile([P, TR, TC], f32, tag="U")
            # slots: q1,q4,q9,q2,q5,q10,q8
            QAB = pool.tile([P, 7, TR, TC], bf16, tag="QAB")
            SS = pool.tile([P, TR, TC], f32, tag="SS")
            RN = pool.tile([P, TR, TC], f32, tag="RN")
            NRM = pool.tile([P, TR, TC], f32, tag="NRM")
            A = pool.tile([P, 3, C, XR, TC], bf16, tag="A")
            # CC slots: C1, C4, C9, C2, C5, C10, C8 | scratch: C5b, C10b
            CC = pool.tile([P, 9, C, TR, TC], bf16, tag="CC")
            # PR slots: P1, P4, P9, P2, P5, P10, P8
            PR = pool.tile([P, 7, C, TR, TC], bf16, tag="PR")
            L1 = pool.tile([P, 3, C, TR, TC], bf16, tag="L1")
            W1 = pool.tile([P, C, TR, TC], bf16, tag="W1")
            W2 = pool.tile([P, C, TR, TC], bf16, tag="W2")
            # F1 sums ~42% of the output mass -> bf16 rounding here is
            # <=0.17% worst-case; only F2 (adds the dominant center term)
            # and OUT stay fp32
            F1 = pool.tile([P, C, TR, TC], bf16, tag="F1")
            F2 = pool.tile([P, C, TR, TC], f32, tag="F2")
            OUTT = pool.tile([P, C, TR, TC], f32, tag="OUTT")

            # GPSIMD-private single-channel tiles (disjoint from DVE's)
            if GPN > 0:
                PRg = pool.tile([P, 7, TR, TC], bf16, tag="PRg")
                L1g = pool.tile([P, 3, TR, TC], bf16, tag="L1g")
                W1g = pool.tile([P, TR, TC], bf16, tag="W1g")
                W2g = pool.tile([P, TR, TC], bf16, tag="W2g")
                F1g = pool.tile([P, TR, TC], bf16, tag="F1g")
                F2g = pool.tile([P, TR, TC], f32, tag="F2g")
                OUTTg = pool.tile([P, TR, TC], f32, tag="OUTTg")

            def gopt(ap):
                o = ap.opt()
                assert len(o.ap) <= 3, (o.shape, list(o.ap))
                return o

            def body():
                # ACT head: V = 2*sigma^2 + eps starts immediately
                nc.scalar.activation(V[:], MD[:], AF.Square,
                                     scale=float(np.sqrt(2.0)), bias=EPS[:])

                # ---- A_t: column pair sums (t-slot, ch, rows incl halo);
                # A1 needs all 22 rows (j=3 shifts); A2 rows 1..20, A3 2..19
                wa = 2 if GPN >= 2 else C
                nc.vector.tensor_tensor(
                    A[:, 0, 0:wa], XO[:, 0:wa, :, 2:2 + TC],
                    XO[:, 0:wa, :, 4:4 + TC], ALU.add)
                nc.vector.tensor_tensor(
                    A[:, 1, 0:wa, 1:21], X[:, 0:wa, 1:21, 2:2 + TC],
                    X[:, 0:wa, 1:21, 6:6 + TC], ALU.add)
                nc.vector.tensor_tensor(
                    A[:, 2, 0:wa, 2:20], XO[:, 0:wa, 2:20, 0:TC],
                    XO[:, 0:wa, 2:20, 6:6 + TC], ALU.add)
                if GPN >= 2:
                    # GPSIMD computes the A-stage for channel 2 into the
                    # SHARED A tile (disjoint slices from DVE's writes)
                    nc.gpsimd.tensor_tensor(
                        gopt(A[:, 0, GC]), gopt(XO[:, GC, :, 2:2 + TC]),
                        gopt(XO[:, GC, :, 4:4 + TC]), ALU.add)
                    nc.gpsimd.tensor_tensor(
                        gopt(A[:, 1, GC, 1:21]),
                        gopt(X[:, GC, 1:21, 2:2 + TC]),
                        gopt(X[:, GC, 1:21, 6:6 + TC]), ALU.add)
                    nc.gpsimd.tensor_tensor(
                        gopt(A[:, 2, GC, 2:20]),
                        gopt(XO[:, GC, 2:20, 0:TC]),
                        gopt(XO[:, GC, 2:20, 6:6 + TC]), ALU.add)

                # ---- per-pixel u = 1/(2 sigma^2 + eps) ----
                nc.vector.reciprocal_approx_fast(U[:], V[:])

                # ---- weight maps on ACT (overlap the pair-sum stage) ----
                for i, m in enumerate((1, 4, 9, 2, 5, 10, 8)):
                    nc.scalar.activation(QAB[:, i], U[:], AF.Exp,
                                         scale=float(-m))

                # ---- X row-pairs (j=1,2,3) in ONE op: the slot axis walks
                # the row offset (stride -TC / +TC elements of the halo-free
                # XN copy) -> CC[0:3]; then += A center rows (C1, C4, C9) ----
                def xslot(j0, slot_stride):
                    b = XN[:, None, :, j0:j0 + TR, :] \
                        .broadcast_to([P, 3, C, TR, TC])
                    ap2 = list(b.ap)
                    ap2[1] = [slot_stride * TC, 3]
                    return BassAP(b.tensor, b.offset, ap2)

                nc.vector.tensor_tensor(
                    CC[:, 0:3], xslot(RHL - 1, -1), xslot(RHL + 1, 1),
                    ALU.add)
                nc.vector.tensor_tensor(
                    CC[:, 0:3], CC[:, 0:3], A[:, :, :, RHL:RHL + TR, :],
                    ALU.add)

                # ---- normalization 1/s^2 = exp(-2 ln s),
                # s = 1+2(q1+q4+q9); recip moves to ACT ----
                nc.vector.tensor_tensor(SS[:], QAB[:, 0], QAB[:, 1], ALU.add)
                nc.vector.tensor_tensor(SS[:], SS[:], QAB[:, 2], ALU.add)
                nc.scalar.activation(RN[:], SS[:], AF.Copy, bias=1.0,
                                     scale=2.0)
                nc.scalar.activation(RN[:], RN[:], AF.Ln)
                nc.scalar.activation(NRM[:], RN[:], AF.Exp, scale=-2.0)

                def arows(sl, j):
                    return (A[:, sl, :, RHL - j:RHL - j + TR, :],
                            A[:, sl, :, RHL + j:RHL + j + TR, :])

                # ---- A row-pairs -> (C2, C5a, C10a) ----
                lo, hi = arows(slice(0, 3), 1)
                nc.vector.tensor_tensor(CC[:, 3:6], lo, hi, ALU.add)
                # j=2 pairs (C5b, C8) written slot-REVERSED so C8 lands at
                # CC[6] (joins the 7-slot product) and C5b at CC[7]
                lo, hi = arows(slice(0, 2), 2)

                def rev2(apv):
                    ap2 = list(apv.ap)
                    sl = ap2[1]
                    assert sl[1] == 2
                    off = apv.offset + sl[0]
                    return BassAP(apv.tensor, off, [ap2[0], [-sl[0], 2]]
                                  + ap2[2:])

                nc.vector.tensor_tensor(rev2(CC[:, 6:8]), lo, hi, ALU.add)
                lo, hi = arows(0, 3)
                nc.vector.tensor_tensor(CC[:, 8], lo, hi, ALU.add)   # C10b
                # C5 += C5b ; C10 += C10b   (contiguous slot pair, one op)
                nc.vector.tensor_tensor(CC[:, 4:6], CC[:, 4:6], CC[:, 7:9],
                                        ALU.add)

                # ======== GPSIMD channel-2 combine suffix ========
                if GPN > 0:
                    nc.gpsimd.tensor_tensor(
                        gopt(PRg[:]), gopt(QAB[:]),
                        gopt(CC[:, 0:7, GC]), ALU.mult)
                    nc.gpsimd.tensor_tensor(
                        gopt(L1g[:]), gopt(PRg[:, 1:4]), gopt(PRg[:, 4:7]),
                        ALU.add)
                    nc.gpsimd.tensor_tensor(
                        gopt(W1g[:]), gopt(L1g[:, 1]), gopt(L1g[:, 2]),
                        ALU.add)
                    nc.gpsimd.tensor_tensor(
                        gopt(W2g[:]), gopt(W1g[:]), gopt(L1g[:, 0]),
                        ALU.add)
                    nc.gpsimd.tensor_tensor(
                        gopt(F1g[:]), gopt(W2g[:]), gopt(PRg[:, 0]),
                        ALU.add)
                    nc.gpsimd.tensor_tensor(
                        gopt(F2g[:]), gopt(F1g[:]),
                        gopt(XN[:, GC, RHL:RHL + TR, :]), ALU.add)
                    nc.gpsimd.tensor_tensor(
                        gopt(OUTTg[:]), gopt(F2g[:]), gopt(NRM[:]),
                        ALU.mult)

                # ======== DVE combine (channels 0..CD) ========
                nc.vector.tensor_tensor(
                    PR[:, :, 0:CD],
                    QAB[:, :, None, :, :].broadcast_to([P, 7, CD, TR, TC]),
                    CC[:, 0:7, 0:CD], ALU.mult)
                nc.vector.tensor_tensor(L1[:, :, 0:CD], PR[:, 1:4, 0:CD],
                                        PR[:, 4:7, 0:CD], ALU.add)
                nc.vector.tensor_tensor(W1[:, 0:CD], L1[:, 1, 0:CD],
                                        L1[:, 2, 0:CD], ALU.add)
                nc.vector.tensor_tensor(W2[:, 0:CD], W1[:, 0:CD],
                                        L1[:, 0, 0:CD], ALU.add)
                nc.vector.tensor_tensor(F1[:, 0:CD], W2[:, 0:CD],
                                        PR[:, 0, 0:CD], ALU.add)
                nc.vector.tensor_tensor(
                    F2[:, 0:CD], F1[:, 0:CD],
                    XN[:, 0:CD, RHL:RHL + TR, :], ALU.add)
                nc.vector.tensor_tensor(
                    OUTT[:, 0:CD], F2[:, 0:CD],
                    NRM[:, None, :, :].broadcast_to([P, CD, TR, TC]),
                    ALU.mult)

            if nrep == 1:
                body()
            else:
                # 16x unrolled hw loop: the per-iteration For_i machinery
                # (~1.4 us) amortizes over 16 serial bodies in timing mode
                UN = 16
                assert nrep % UN == 0, nrep
                with tc.For_i(0, nrep // UN, 1):
                    for _ in range(UN):
                        body()
            nc.sync.dma_start(out=out[:], in_=OUTT[:])
            if GPN > 0:
                nc.sync.dma_start(out=outg[:], in_=OUTTg[:])
            else:
                nc.sync.dma_start(out=outg[:], in_=OUTT[:, C - 1])

    nc.compile()
    _NC_CACHE[key] = nc
    return nc


def _stage_inputs(img, modulator):
    import ml_dtypes
    x = np.ascontiguousarray(np.asarray(img, dtype=np.float32))[0]  # (3,H,W)
    mod = np.ascontiguousarray(np.asarray(modulator, dtype=np.float32))
    xpad = np.pad(x, ((0, 0), (RHL, RHL), (CHL, CHL + 1)), mode="edge")
    # (3, 518, 521)
    idx_r = (np.arange(RQ) * TR)[:, None] + np.arange(XR)[None, :]  # (4,22)
    idx_c = (np.arange(CB) * TC)[:, None] + np.arange(XC)[None, :]  # (32,24)
    idx_cn = (np.arange(CB) * TC)[:, None] + CHL + np.arange(TC)[None, :]
    mir = (np.arange(RQ) * TR)[:, None] + np.arange(TR)[None, :]
    mic = (np.arange(CB) * TC)[:, None] + np.arange(TC)[None, :]
    in_maps = []
    for core in range(NC):
        sub = xpad[:, core * RS:core * RS + RS + 2 * RHL, :]  # (3,70,521)
        # (3, 4, 32, 22, 24) -> (128, 3, 22, 24)
        blk = sub[:, idx_r[:, None, :, None], idx_c[None, :, None, :]]
        xt = np.ascontiguousarray(
            blk.transpose(1, 2, 0, 3, 4).reshape(P, C, XR, XC)
        ).astype(ml_dtypes.bfloat16)
        blk_o = sub[:, idx_r[:, None, :, None], idx_c[None, :, None, :] + 1]
        xot = np.ascontiguousarray(
            blk_o.transpose(1, 2, 0, 3, 4).reshape(P, C, XR, XC)
        ).astype(ml_dtypes.bfloat16)
        blk_n = sub[:, idx_r[:, None, :, None], idx_cn[None, :, None, :]]
        xnt = np.ascontiguousarray(
            blk_n.transpose(1, 2, 0, 3, 4).reshape(P, C, XR, TC)
        ).astype(ml_dtypes.bfloat16)
        msub = mod[core * RS:core * RS + RS, :]  # (64, 512)
        mdt = np.ascontiguousarray(
            msub[mir[:, None, :, None], mic[None, :, None, :]]
            .reshape(P, TR, TC))
        in_maps.append({"x": xt, "xo": xot, "xn": xnt, "md": mdt})
    return in_maps


def kernel(img, modulator):
    from concourse.bass_utils import run_bass_kernel_spmd

    nc = _build_nc()
    in_maps = _stage_inputs(img, modulator)
    res = run_bass_kernel_spmd(nc, in_maps, list(range(NC))).results
    use_g = _gpn() > 0
    # per-core out [128, 3, 16, 16] -> (3, 64, 512)
    parts = []
    for i in range(NC):
        o = np.asarray(res[i]["out"]).reshape(RQ, CB, C, TR, TC).copy()
        if use_g:
            og = np.asarray(res[i]["outg"]).reshape(RQ, CB, TR, TC)
            o[:, :, C - 1] = og
        parts.append(o.transpose(2, 0, 3, 1, 4).reshape(C, RS, W))
    out = np.concatenate(parts, axis=1)
    return np.ascontiguousarray(out[None], dtype=np.float32)


# revision 5
# speedup vs baseline: 1.3435x; 1.3435x over previous
"""LocalGaussianBlur v4 — Trainium2 Bass kernel (7x7 truncation, fused DVE).

Math: sigma = modulator[h,w] in (0,1); u = 1/(2 sigma^2 + 1e-8);
q = exp(-u) <= exp(-0.5).  Weight of tap (j,t) is q^(j^2+t^2).
Since q <= 0.6065, taps with |j| or |t| >= 4 carry < 5.4e-4 of the
kernel mass; normalizing by the truncated sum s3 = 1 + 2(q+q^4+q^9)
makes the truncated kernel a proper weighted average.  Rings kept:
m in {1,2,4,5,8,9,10} (13/18 dropped).

out = [Xc + q C1 + q^2 C2 + q^4 C4 + q^5 C5 + q^8 C8 + q^9 C9
        + q^10 C10] / s3^2

Layout per core (8-way H-shard, 64 rows each):
  partitions p = rq*32+cb (4 row-quarters x 32 col-blocks),
  per-partition spatial block 16x16.  The image is staged as XS
  [128, 7, 3ch, 22, 16] bf16: seven column-shifted copies (shift s-3)
  of the 16-col block with 3 halo rows.  Pre-shifting bakes every
  column offset into contiguous, 4B-aligned rows, so all DVE ops run
  in the 2x bf16 perf mode, and the whole t=+-1,2,3 column-pair stage
  collapses to ONE tensor_tensor (slot axis walks the shift).

v4 fusions (vs v2's 19 DVE ops -> 15):
 - A-stage: 3 ops -> 1 via the XS slot axis ([P,3,1056] APs).
 - ring stage: CC36/rev2/CC8/merge -> grid/M10/merge.  The grid op's
   free dims are [j=1,2][sl=A1,A2][ch][256]: one op yields C2, C5a,
   C5b, C8 with slot placement solved so PR later reads the 7 ring
   maps contiguously in QAB order; M10 pairs (A3[r-1],A1[r-3]) etc.
 - tree: W1/W2/F1 -> V/F1 by placing L1 (3 slots) + P1 in one tile so
   (L1a+L1b, L1c+P1) is a single stride-2-slot op.
Weight maps (ACT) and norm chain unchanged.  LGB2_DMAX=1 additionally
moves the X row-pair op (xslot) onto the DMA engines as a SWDGE
copy + accumulate-DMA pair (CCE inline add), freeing ~1.3us of DVE.

CCA slot map  [P, 15, C, TR, TC]:
  3=C5a | 4=C1 5=C4 6=C9 7=C2 8=C5 9=C10 10=C8 | 12=M10a 13=M10b 14=C5b
  PR window = slots [4:11) in QAB exp order (1,4,9,2,5,10,8).
  grid out (C2@7, C5a@3, C5b@14, C8@10) is affine: j-stride +7 slots,
  sl-stride -4 slots.  merge: (C5,C10) = (C5a,M10a)+(C5b,M10b) with
  in0 slot-stride +9, in1 slot-stride -1.
"""

import os
import numpy as np

H = W = 512
C = 3
NC = 8
RS = H // NC        # 64 rows per core
RQ = 4              # row-quarters per core
CB = 32             # col blocks
TR = 16             # block rows
TC = 16             # block cols
RHL = 3             # row halo
XR = TR + 2 * RHL   # 22
NS = 7              # staged column shifts (-3..3)
P = 128

_NC_CACHE = {}


def _build_nc():
    key = ("nc", os.environ.get("LGB2_REPEAT", "1"),
           os.environ.get("LGB2_DMAX", "0"))
    if key in _NC_CACHE:
        return _NC_CACHE[key]
    import concourse.bass as bass  # noqa: F401
    from concourse import bacc
    import concourse.mybir as mybir
    from concourse.tile import TileContext
    from concourse.ap import AP as BassAP

    f32 = mybir.dt.float32
    bf16 = mybir.dt.bfloat16
    AF = mybir.ActivationFunctionType
    ALU = mybir.AluOpType

    nc = bacc.Bacc()
    xs = nc.dram_tensor("xs", [P, NS, C, XR, TC], bf16, kind="ExternalInput")
    md = nc.dram_tensor("md", [P, TR, TC], f32, kind="ExternalInput")
    out = nc.dram_tensor("out", [P, C, TR, TC], f32, kind="ExternalOutput")

    nrep = int(os.environ.get("LGB2_REPEAT", "1"))
    DMAX = os.environ.get("LGB2_DMAX", "0") == "1"

    DCC = C * TR * TC          # CCA slot stride (elements)
    DA = C * XR * TC           # A slot stride
    DXS = C * XR * TC          # XS slot stride

    with TileContext(nc) as tc:
        with tc.tile_pool(name="main", bufs=1) as pool:
            XS = pool.tile([P, NS, C, XR, TC], bf16, tag="XS")
            MD = pool.tile([P, TR, TC], f32, tag="MD")
            nc.sync.dma_start(out=MD[:], in_=md[:])
            nc.sync.dma_start(out=XS[:], in_=xs[:])
            EPS = pool.tile([P, 1], f32, tag="EPS")
            nc.vector.memset(EPS[:], 1e-4)

            V = pool.tile([P, TR, TC], f32, tag="V")
            U = pool.tile([P, TR, TC], f32, tag="U")
            # slots: q1,q4,q9,q2,q5,q10,q8
            QAB = pool.tile([P, 7, TR, TC], bf16, tag="QAB")
            SS = pool.tile([P, TR, TC], f32, tag="SS")
            RN = pool.tile([P, TR, TC], f32, tag="RN")
            NRM = pool.tile([P, TR, TC], f32, tag="NRM")
            # A slots: t=1,2,3 column-pair sums (all 22 rows)
            A = pool.tile([P, 3, C, XR, TC], bf16, tag="A")
            CCA = pool.tile([P, 15, C, TR, TC], bf16, tag="CCA")
            # PRX slots: L1a,L1b,L1c,P1,P4,P9,P2,P5,P10,P8
            PRX = pool.tile([P, 10, C, TR, TC], bf16, tag="PRX")
            V2 = pool.tile([P, 2, C, TR, TC], bf16, tag="V2")
            F1 = pool.tile([P, C, TR, TC], bf16, tag="F1")
            F2 = pool.tile([P, C, TR, TC], f32, tag="F2")
            OUTT = pool.tile([P, C, TR, TC], f32, tag="OUTT")

            XN = XS[:, 3]                   # center copy [P, C, XR, TC]

            def slotap(tile_ap, slots_stride, n, extra=None):
                """AP over `tile_ap` with dim1 replaced by [stride, n]."""
                ap2 = list(tile_ap.ap)
                ap2[1] = [slots_stride, n]
                return BassAP(tile_ap.tensor, tile_ap.offset, ap2)

            def body():
                # ACT head: V = 2*sigma^2 + eps starts immediately
                nc.scalar.activation(V[:], MD[:], AF.Square,
                                     scale=float(np.sqrt(2.0)), bias=EPS[:])

                # ---- A_t = X(c-t)+X(c+t), t=1,2,3, ONE op: in0 walks XS
                # slots 2,1,0 (stride -DXS), in1 slots 4,5,6 (stride +DXS)
                in0 = slotap(XS[:, 2:5], -DXS, 3)
                in1 = slotap(XS[:, 4:7], DXS, 3)
                nc.vector.tensor_tensor(A[:], in0, in1, ALU.add)

                # ---- per-pixel u = 1/(2 sigma^2 + eps) ----
                nc.vector.reciprocal_approx_fast(U[:], V[:])

                # ---- weight maps on ACT (overlap the pair-sum stage) ----
                for i, m in enumerate((1, 4, 9, 2, 5, 10, 8)):
                    nc.scalar.activation(QAB[:, i], U[:], AF.Exp,
                                         scale=float(-m))

                # ---- X row-pairs j=1,2,3 -> (C1,C4,C9) = CCA[4:7) ----
                def xpair(j0, slot_stride):
                    b = XN[:, None, :, j0:j0 + TR, :] \
                        .broadcast_to([P, 3, C, TR, TC])
                    ap2 = list(b.ap)
                    ap2[1] = [slot_stride * TC, 3]
                    return BassAP(b.tensor, b.offset, ap2)

                if DMAX:
                    nc.gpsimd.dma_start(out=CCA[:, 4:7],
                                        in_=xpair(RHL - 1, -1))
                    nc.gpsimd.dma_start(out=CCA[:, 4:7],
                                        in_=xpair(RHL + 1, 1),
                                        accum_op=ALU.add)
                else:
                    nc.vector.tensor_tensor(CCA[:, 4:7], xpair(RHL - 1, -1),
                                            xpair(RHL + 1, 1), ALU.add)

                # ---- normalization 1/s^2 = exp(-2 ln s) ----
                nc.vector.tensor_tensor(SS[:], QAB[:, 0], QAB[:, 1], ALU.add)
                nc.vector.tensor_tensor(SS[:], SS[:], QAB[:, 2], ALU.add)
                nc.scalar.activation(RN[:], SS[:], AF.Copy, bias=1.0,
                                     scale=2.0)
                nc.scalar.activation(RN[:], RN[:], AF.Ln)
                nc.scalar.activation(NRM[:], RN[:], AF.Exp, scale=-2.0)

                # ---- ring grid op: out[j][sl] = A[sl][r-j] + A[sl][r+j]
                # for j in {1,2} x sl in {A1,A2} -> C2@7, C5a@3, C5b@14,
                # C8@10 (j-stride +7 slots, sl-stride -4 slots) ----
                def gr_in(sign):
                    # dims [j(rowoff -TC), sl(+DA), C, 256]
                    r0 = RHL - sign * 1
                    o = A[:, 0:2, :, r0:r0 + TR, :]   # placeholder shape
                    ap2 = list(o.ap)
                    # [P][sl][C][TR][TC] -> rebuild: dim1=j, dim2=sl, dim3=C,
                    # dim4=rows*cols contiguous
                    base = A[:, 0, 0, r0, 0]
                    ap = [list(o.ap)[0],
                          [-sign * TC, 2],        # j: rows shift by -+1,2
                          [DA, 2],                # sl: A1, A2
                          [XR * TC, C],           # channel
                          [1, TR * TC]]           # 16 contiguous rows
                    return BassAP(o.tensor, base.offset, ap)

                def gr_out():
                    base = CCA[:, 7, 0, 0, 0]     # C2 slot
                    ap = [list(CCA[:].ap)[0],
                          [7 * DCC, 2],           # j stride: +7 slots
                          [-4 * DCC, 2],          # sl stride: -4 slots
                          [TR * TC, C],
                          [1, TR * TC]]
                    return BassAP(CCA[:].tensor, base.offset, ap)

                nc.vector.tensor_tensor(gr_out(), gr_in(+1), gr_in(-1),
                                        ALU.add)

                # ---- M10: (M10a, M10b) = (A3[r-1]+A3[r+1] , A1[r-3]+
                # A1[r+3]) -- wait: M10a = A3[r-1]+A3[r+1] is rowpair1(A3),
                # M10b = rowpair3(A1); written to CCA slots 12, 13 ----
                def m10_in(sign):
                    # slots: (A3 rows RHL-+1, A1 rows RHL-+3)
                    base = A[:, 2, 0, RHL - sign * 1, 0]
                    stride = (0 * DA + (RHL - sign * 3) * TC) \
                        - (2 * DA + (RHL - sign * 1) * TC)
                    ap = [list(A[:].ap)[0],
                          [stride, 2],
                          [XR * TC, C],
                          [1, TR * TC]]
                    return BassAP(A[:].tensor, base.offset, ap)

                def cca_slots(s0, stride, n):
                    base = CCA[:, s0, 0, 0, 0]
                    ap = [list(CCA[:].ap)[0],
                          [stride * DCC, n],
                          [TR * TC, C],
                          [1, TR * TC]]
                    return BassAP(CCA[:].tensor, base.offset, ap)

                nc.vector.tensor_tensor(cca_slots(12, 1, 2), m10_in(+1),
                                        m10_in(-1), ALU.add)

                # ---- CCA[4:7) += A center rows (C1+=A1c, C4+=A2c, C9+=A3c)
                nc.vector.tensor_tensor(
                    CCA[:, 4:7], CCA[:, 4:7],
                    A[:, :, :, RHL:RHL + TR, :], ALU.add)

                # ---- merge: (C5@8, C10@9) = (C5a@3, M10a@12) + (C5b@14,
                # M10b@13): in0 slot-stride +9, in1 slot-stride -1 ----
                nc.vector.tensor_tensor(cca_slots(8, 1, 2),
                                        cca_slots(3, 9, 2),
                                        cca_slots(14, -1, 2), ALU.add)

                # ---- products into PRX[3:10) ----
                nc.vector.tensor_tensor(
                    PRX[:, 3:10],
                    QAB[:, :, None, :, :].broadcast_to([P, 7, C, TR, TC]),
                    CCA[:, 4:11], ALU.mult)
                # L1 = (P4+P2, P9+P5, P10+P8) -> PRX[0:3)
                nc.vector.tensor_tensor(PRX[:, 0:3], PRX[:, 4:7],
                                        PRX[:, 7:10], ALU.add)
                # V2 = (L1a+L1b, L1c+P1): stride-2-slot pairs
                def prx_slots(s0, stride, n):
                    base = PRX[:, s0, 0, 0, 0]
                    ap = [list(PRX[:].ap)[0],
                          [stride * DCC, n],
                          [TR * TC, C],
                          [1, TR * TC]]
                    return BassAP(PRX[:].tensor, base.offset, ap)

                nc.vector.tensor_tensor(V2[:], prx_slots(0, 2, 2),
                                        prx_slots(1, 2, 2), ALU.add)
                nc.vector.tensor_tensor(F1[:], V2[:, 0], V2[:, 1], ALU.add)
                nc.vector.tensor_tensor(
                    F2[:], F1[:], XN[:, :, RHL:RHL + TR, :], ALU.add)
                nc.vector.tensor_tensor(
                    OUTT[:], F2[:],
                    NRM[:, None, :, :].broadcast_to([P, C, TR, TC]),
                    ALU.mult)

            if nrep == 1:
                body()
            else:
                # 16x unrolled hw loop: the per-iteration For_i machinery
                # (~1.4 us) amortizes over 16 serial bodies in timing mode
                UN = 16
                assert nrep % UN == 0, nrep
                with tc.For_i(0, nrep // UN, 1):
                    for _ in range(UN):
                        body()
            nc.sync.dma_start(out=out[:], in_=OUTT[:])

    nc.compile()
    _NC_CACHE[key] = nc
    return nc


def _stage_inputs(img, modulator):
    import ml_dtypes
    x = np.ascontiguousarray(np.asarray(img, dtype=np.float32))[0]  # (3,H,W)
    mod = np.ascontiguousarray(np.asarray(modulator, dtype=np.float32))
    xpad = np.pad(x, ((0, 0), (RHL, RHL), (3, 3)), mode="edge")
    # (3, 518, 518)
    idx_r = (np.arange(RQ) * TR)[:, None] + np.arange(XR)[None, :]  # (4,22)
    mir = (np.arange(RQ) * TR)[:, None] + np.arange(TR)[None, :]
    mic = (np.arange(CB) * TC)[:, None] + np.arange(TC)[None, :]
    in_maps = []
    for core in range(NC):
        sub = xpad[:, core * RS:core * RS + RS + 2 * RHL, :]  # (3,70,518)
        # XS[p=(rq,cb), s, c, r, k] = sub[c, rq*16+r, cb*16+k+s]
        # (3, 4, 22, 32, 7+16-1 window) via strided gather:
        idx_c = (np.arange(CB) * TC)[:, None, None] \
            + np.arange(NS)[None, :, None] + np.arange(TC)[None, None, :]
        # (32, 7, 16); col index = cb*16 + s + k  (shift s-3 after -3 pad)
        blk = sub[:, idx_r[:, None, :, None, None],
                  idx_c[None, :, None, :, :]]          # (3,4,32,22,7,16)
        xst = np.ascontiguousarray(
            blk.transpose(1, 2, 4, 0, 3, 5).reshape(P, NS, C, XR, TC)
        ).astype(ml_dtypes.bfloat16)
        msub = mod[core * RS:core * RS + RS, :]  # (64, 512)
        mdt = np.ascontiguousarray(
            msub[mir[:, None, :, None], mic[None, :, None, :]]
            .reshape(P, TR, TC))
        in_maps.append({"xs": xst, "md": mdt})
    return in_maps


def kernel(img, modulator):
    from concourse.bass_utils import run_bass_kernel_spmd

    nc = _build_nc()
    in_maps = _stage_inputs(img, modulator)
    res = run_bass_kernel_spmd(nc, in_maps, list(range(NC))).results
    # per-core out [128, 3, 16, 16] -> (3, 64, 512)
    parts = []
    for i in range(NC):
        o = np.asarray(res[i]["out"]).reshape(RQ, CB, C, TR, TC)
        parts.append(o.transpose(2, 0, 3, 1, 4).reshape(C, RS, W))
    out = np.concatenate(parts, axis=1)
    return np.ascontiguousarray(out[None], dtype=np.float32)
